# revision 13
# baseline (speedup 1.0000x reference)
"""Trainium2 Bass kernel for nn_AutoconstraintModel (gnn_message_passing).

Model (per reference):
  edge_partner:  logits[n] = Wp2.T @ relu(Wp1.T @ [cur_g(n); x_n; glob_g(n)] + bp1) + bp2
  edge_label:    3-layer MLP on [cur_b; partner_b; glob_b] per graph  -> [B, 14]

Strategy (data-parallel over graphs, 8 cores, node ranges aligned to graph
boundaries):
  - Host pre-transposes the node embeddings so each core reads xT = [D, Ns]
    (features on partitions) with perfectly contiguous DMA runs.
  - The per-graph part of layer 1 (cur/glob contributions + bp1) is a
    per-graph constant vector cvec[:, g], computed once on-device via two
    matmuls; the per-node matmul then only contracts the node embedding
    (K=128 instead of 384).
  - Per 512-node chunk: PE matmul (Wp1_node), DVE broadcast-add of cvec
    (graph-aligned, step-0 AP), ACT relu, PE matmul (Wp2) into a shared
    PSUM logit accumulator at partition 32*j via tile_position, extraction
    alternates DVE/ACT to balance engine load.
"""

import os
import sys

import numpy as np

for _p in ("/opt/trn_rl_repo", "/root/.axon_site/_ro/trn_rl_repo"):
    if os.path.isdir(_p) and _p not in sys.path:
        sys.path.append(_p)

# ---- static problem dimensions (hardcoded per contest contract) ----
B = 8192          # graphs
D = 128           # embedding dim
NL = 14           # label logits
NPG = 64          # nodes per graph
N = B * NPG       # 524288 nodes
M = 8             # cores
BS = B // M       # 1024 graphs per core
NS = N // M       # 65536 nodes per core
CH = 512          # nodes per compute chunk (8 graphs)
SUPER = 2048      # nodes per input DMA (1 MiB)
NSUPER = NS // SUPER          # 32
CPS = SUPER // CH             # chunks per super (4)

_cache = {}


def _split_multi_waits(nc, max_waits=1):
    """This walrus build rejects instructions carrying more than one sem
    wait. Hoist extra waits onto same-engine NoOps inserted just before the
    instruction (the engine sequencer executes them in program order, so the
    semantics are identical)."""
    import bass_rust
    import concourse.mybir as mybir

    for fn in nc.m.functions:
        for blk in fn.blocks:
            new_list = []
            changed = False
            for inst in blk.instructions:
                si = inst.sync_info
                waits = list(si.on_wait) if si and si.on_wait else []
                if len(waits) > max_waits:
                    changed = True
                    extra, keep = waits[:-max_waits], waits[-max_waits:]
                    for i in range(0, len(extra), max_waits):
                        nop = mybir.InstNoOp(
                            name=nc.get_next_instruction_name(), ins=[],
                            outs=[])
                        nop.engine = inst.engine
                        nop.sync_info = bass_rust.SyncInfo(
                            on_wait=extra[i:i + max_waits], on_update=[])
                        nc.register_instruction(nop)
                        new_list.append(nop)
                    si.on_wait = keep
                new_list.append(inst)
            if changed:
                blk.instructions = new_list


def _build(repeat=1):
    """Build the per-core Bass program. `repeat` re-traces the whole body
    that many times (used only for timing runs)."""
    import concourse.bass as bass
    import concourse.mybir as mybir
    import concourse.tile as tile

    f32 = mybir.dt.float32
    AF = mybir.ActivationFunctionType
    OP = mybir.AluOpType

    nc = bass.Bass()
    dp = nc.declare_dram_parameter
    xT = dp("xt", [D, NS], f32, isOutput=False)
    curT = dp("curt", [D, BS], f32, isOutput=False)
    globT = dp("globt", [D, BS], f32, isOutput=False)
    lcurT = dp("lcurt", [D, BS], f32, isOutput=False)
    lpartT = dp("lpartt", [D, BS], f32, isOutput=False)
    lglobT = dp("lglobt", [D, BS], f32, isOutput=False)
    wp1c = dp("wp1c", [D, D], f32, isOutput=False)
    wp1n = dp("wp1n", [D, D], f32, isOutput=False)
    wp1g = dp("wp1g", [D, D], f32, isOutput=False)
    bp1 = dp("bp1", [D, 1], f32, isOutput=False)
    wp2 = dp("wp2", [D, 32], f32, isOutput=False)   # Wp2 replicated 32x
    bp2r = dp("bp2r", [D, 1], f32, isOutput=False)
    wl1c = dp("wl1c", [D, D], f32, isOutput=False)
    wl1p = dp("wl1p", [D, D], f32, isOutput=False)
    wl1g = dp("wl1g", [D, D], f32, isOutput=False)
    bl1 = dp("bl1", [D, 1], f32, isOutput=False)
    wl2 = dp("wl2", [D, D], f32, isOutput=False)
    bl2 = dp("bl2", [D, 1], f32, isOutput=False)
    wl3 = dp("wl3", [D, NL], f32, isOutput=False)
    bl3 = dp("bl3", [NL, 1], f32, isOutput=False)
    out_pl = dp("out_pl", [NS], f32, isOutput=True)
    out_ll = dp("out_ll", [NL, BS], f32, isOutput=True)

    with tile.TileContext(nc) as tc:
        with (
            tc.tile_pool(name="weights", bufs=1) as wpool,
            tc.tile_pool(name="aux", bufs=1) as apool,
            tc.tile_pool(name="xin", bufs=4) as xpool,
            tc.tile_pool(name="hps", bufs=3, space="PSUM") as hpsum,
            tc.tile_pool(name="llps", bufs=2, space="PSUM") as llpsum,
            tc.tile_pool(name="lps", bufs=2, space="PSUM") as lpsum,
            tc.tile_pool(name="tsb", bufs=4) as spool,
            tc.tile_pool(name="rsb", bufs=4) as rpool,
            tc.tile_pool(name="osb", bufs=4) as opool,
        ):
            def load(pool, ap, shape):
                t = pool.tile(shape, f32, tag=f"in_{ap.name}")
                nc.sync.dma_start(t[:], ap[:])
                return t

            wp1c_sb = load(wpool, wp1c, [D, D])
            wp1n_sb = load(wpool, wp1n, [D, D])
            wp1g_sb = load(wpool, wp1g, [D, D])
            bp1_sb = load(wpool, bp1, [D, 1])
            wp2_sb = load(wpool, wp2, [D, 32])
            bp2r_sb = load(wpool, bp2r, [D, 1])
            wl1c_sb = load(wpool, wl1c, [D, D])
            wl1p_sb = load(wpool, wl1p, [D, D])
            wl1g_sb = load(wpool, wl1g, [D, D])
            bl1_sb = load(wpool, bl1, [D, 1])
            wl2_sb = load(wpool, wl2, [D, D])
            bl2_sb = load(wpool, bl2, [D, 1])
            wl3_sb = load(wpool, wl3, [D, NL])
            bl3_sb = load(wpool, bl3, [NL, 1])
            curT_sb = load(apool, curT, [D, BS])
            globT_sb = load(apool, globT, [D, BS])
            lcurT_sb = load(apool, lcurT, [D, BS])
            lpartT_sb = load(apool, lpartT, [D, BS])
            lglobT_sb = load(apool, lglobT, [D, BS])

            for _rep in range(repeat):
                # ---- per-graph constant of partner layer 1 ----
                cvec_sb = apool.tile([D, BS], f32, tag="cvec")
                for h in range(BS // 512):
                    sl = slice(512 * h, 512 * (h + 1))
                    ps = hpsum.tile([D, 512], f32, tag="hps")
                    nc.tensor.matmul(ps[:], wp1c_sb[:], curT_sb[:, sl],
                                     start=True, stop=False)
                    nc.tensor.matmul(ps[:], wp1g_sb[:], globT_sb[:, sl],
                                     start=False, stop=True)
                    nc.scalar.activation(cvec_sb[:, sl], ps[:], AF.Identity,
                                         bias=bp1_sb[:])

                # ---- label head ----
                h1_sb = apool.tile([D, BS], f32, tag="h1")
                h2_sb = apool.tile([D, BS], f32, tag="h2")
                ll_sb = apool.tile([NL, BS], f32, tag="ll")
                for h in range(BS // 512):
                    sl = slice(512 * h, 512 * (h + 1))
                    ps = hpsum.tile([D, 512], f32, tag="hps")
                    nc.tensor.matmul(ps[:], wl1c_sb[:], lcurT_sb[:, sl],
                                     start=True, stop=False)
                    nc.tensor.matmul(ps[:], wl1p_sb[:], lpartT_sb[:, sl],
                                     start=False, stop=False)
                    nc.tensor.matmul(ps[:], wl1g_sb[:], lglobT_sb[:, sl],
                                     start=False, stop=True)
                    nc.scalar.activation(h1_sb[:, sl], ps[:], AF.Relu,
                                         bias=bl1_sb[:])
                for h in range(BS // 512):
                    sl = slice(512 * h, 512 * (h + 1))
                    ps = hpsum.tile([D, 512], f32, tag="hps")
                    nc.tensor.matmul(ps[:], wl2_sb[:], h1_sb[:, sl],
                                     start=True, stop=True)
                    nc.scalar.activation(h2_sb[:, sl], ps[:], AF.Relu,
                                         bias=bl2_sb[:])
                for h in range(BS // 512):
                    sl = slice(512 * h, 512 * (h + 1))
                    ps = llpsum.tile([NL, 512], f32, tag="llps")
                    nc.tensor.matmul(ps[:], wl3_sb[:], h2_sb[:, sl],
                                     start=True, stop=True)
                    nc.scalar.activation(ll_sb[:, sl], ps[:], AF.Identity,
                                         bias=bl3_sb[:])
                nc.sync.dma_start(out_ll[:, :], ll_sb[:])

                # ---- main loop: partner logits over all nodes ----
                for sc in range(NSUPER):
                    xt = xpool.tile([D, SUPER], f32, tag="xt")
                    nc.sync.dma_start(
                        xt[:], xT[:, sc * SUPER:(sc + 1) * SUPER])
                    lg_ps = lpsum.tile([D, 512], f32, tag="lg")
                    for j4 in range(CPS):
                        j = CPS * sc + j4          # global chunk id
                        g0 = 8 * j                 # first graph of chunk
                        h_ps = hpsum.tile([D, CH], f32, tag="hps")
                        nc.tensor.matmul(h_ps[:], wp1n_sb[:],
                                         xt[:, CH * j4:CH * (j4 + 1)],
                                         start=True, stop=True)
                        t_sb = spool.tile([D, CH], f32, tag="t")
                        nc.vector.tensor_tensor(
                            t_sb[:].rearrange("p (g r) -> p g r", g=8),
                            h_ps[:].rearrange("p (g r) -> p g r", g=8),
                            cvec_sb[:, g0:g0 + 8].unsqueeze(2)
                                .broadcast_to([D, 8, NPG]),
                            OP.add,
                        )
                        r_sb = rpool.tile([D, CH], f32, tag="r")
                        nc.scalar.activation(r_sb[:], t_sb[:], AF.Relu)
                        nc.tensor.matmul(
                            lg_ps[32 * j4:32 * (j4 + 1), :], wp2_sb[:],
                            r_sb[:], start=True, stop=True,
                            tile_position=(0, 32 * j4))
                    lo = opool.tile([D, 512], f32, tag="lo")
                    if sc % 2 == 0:
                        nc.vector.tensor_scalar_add(lo[:], lg_ps[:],
                                                    bp2r_sb[:])
                    else:
                        nc.scalar.activation(lo[:], lg_ps[:], AF.Identity,
                                             bias=bp2r_sb[:])
                    nc.sync.dma_start(
                        out_pl[bass.ds(sc * SUPER, SUPER)]
                        .rearrange("(p f) -> p f", p=CPS),
                        lo[0:128:32, :],
                    )
    _split_multi_waits(nc)
    return nc


def _get_nc(repeat=1):
    key = ("nc", repeat)
    if key not in _cache:
        _cache[key] = _build(repeat)
    return _cache[key]


def _host_prep(inputs):
    """Shard + lay out inputs for the 8 cores. Returns list of in_maps."""
    x = np.ascontiguousarray(inputs["node_post_embedding"], dtype=np.float32)
    ge = np.ascontiguousarray(inputs["global_embedding"], dtype=np.float32)
    pgi = np.asarray(inputs["partner_graph_idx"])
    pni = np.asarray(inputs["partner_node_idx"])

    cur = x[NPG - 1::NPG]                    # [B, D] current node per graph
    lcur = cur[pgi]                          # label-head gathers
    lglob = ge[pgi]
    lpart = x[pni]

    wp1 = np.asarray(inputs["Wp1"], dtype=np.float32)
    wl1 = np.asarray(inputs["Wl1"], dtype=np.float32)
    rep = {
        "wp1c": np.ascontiguousarray(wp1[0:D]),
        "wp1n": np.ascontiguousarray(wp1[D:2 * D]),
        "wp1g": np.ascontiguousarray(wp1[2 * D:3 * D]),
        "bp1": np.asarray(inputs["bp1"], np.float32).reshape(D, 1),
        "wp2": np.ascontiguousarray(
            np.tile(np.asarray(inputs["Wp2"], np.float32).reshape(D, 1),
                    (1, 32))),
        "bp2r": np.full((D, 1), np.asarray(inputs["bp2"], np.float32)[0],
                        np.float32),
        "wl1c": np.ascontiguousarray(wl1[0:D]),
        "wl1p": np.ascontiguousarray(wl1[D:2 * D]),
        "wl1g": np.ascontiguousarray(wl1[2 * D:3 * D]),
        "bl1": np.asarray(inputs["bl1"], np.float32).reshape(D, 1),
        "wl2": np.ascontiguousarray(np.asarray(inputs["Wl2"], np.float32)),
        "bl2": np.asarray(inputs["bl2"], np.float32).reshape(D, 1),
        "wl3": np.ascontiguousarray(np.asarray(inputs["Wl3"], np.float32)),
        "bl3": np.asarray(inputs["bl3"], np.float32).reshape(NL, 1),
    }

    in_maps = []
    for k in range(M):
        ns, gs = slice(k * NS, (k + 1) * NS), slice(k * BS, (k + 1) * BS)
        m = dict(rep)
        m["xt"] = np.ascontiguousarray(x[ns].T)
        m["curt"] = np.ascontiguousarray(cur[gs].T)
        m["globt"] = np.ascontiguousarray(ge[gs].T)
        m["lcurt"] = np.ascontiguousarray(lcur[gs].T)
        m["lpartt"] = np.ascontiguousarray(lpart[gs].T)
        m["lglobt"] = np.ascontiguousarray(lglob[gs].T)
        in_maps.append(m)
    return in_maps


def _gather(results):
    pl = np.concatenate([np.asarray(r["out_pl"]) for r in results])
    ll = np.concatenate(
        [np.asarray(r["out_ll"]).T for r in results], axis=0)
    return pl.astype(np.float32), ll.astype(np.float32)


def kernel(**inputs):
    from concourse.bass_utils import run_bass_kernel_spmd

    nc = _get_nc(repeat=1)
    in_maps = _host_prep(inputs)
    res = run_bass_kernel_spmd(nc, in_maps, list(range(M)))
    return _gather(res.results)


def run_timed(inputs, repeats=(1, 4), reps=3):
    """Timing helper for test.py: returns (outputs, est_exec_ns, raw_timings).

    Runs the repeat-R build; per-iteration HW time estimated from the wall
    time difference between the R=repeats[1] and R=repeats[0] builds with
    device-resident inputs.
    """
    import time

    import jax
    from concourse import bass2jax

    in_maps = _host_prep(inputs)
    walls = {}
    outputs = None
    for rpt in repeats:
        nc = _get_nc(repeat=rpt)
        # build the jitted runner once per repeat count
        import timing as _t  # local helper living next to test.py
        fn, pack, out_names, out_avals = _t.make_runner(nc, M, 1)
        concat_in, concat_zeros = pack(in_maps)
        dev_in = [jax.device_put(a) for a in concat_in]
        best = None
        for r in range(reps + 1):
            zeros = [jax.device_put(z) for z in concat_zeros]
            t0 = time.perf_counter()
            outs = fn(*dev_in, *zeros)
            jax.block_until_ready(outs)
            dt = time.perf_counter() - t0
            if r > 0:
                best = dt if best is None else min(best, dt)
        walls[rpt] = best
        if rpt == repeats[0] and outputs is None:
            results = [
                {name: np.asarray(outs[i]).reshape(M, *out_avals[i].shape)[c]
                 for i, name in enumerate(out_names)}
                for c in range(M)
            ]
            outputs = _gather(results)
    r0, r1 = repeats[0], repeats[-1]
    est_ns = (walls[r1] - walls[r0]) / (r1 - r0) * 1e9
    return outputs, est_ns, walls
